# revision 35
# baseline (speedup 1.0000x reference)
"""Trainium2 Bass kernel for nn_AttentionConv2D (two conv3x3+BN branches with
position-aware attention maps), SPMD over 8 NeuronCores.

Sharding: core = batch_index * 2 + h_half. Each core computes both branches for
one batch element's 128-row horizontal slab (plus 1-row halo for the attention
3x3 conv). All cross-core data movement is done host-side (overlapping input
row slices, per-core position/band constants); the device program is identical
on every core.

v2 layout (bf16 input path, fp32 accumulation):
  conv3x3 (both branches, 128 out ch) -> implicit GEMM, 9 taps x row-pair
    matmuls (bf16, N=512) accumulated in PSUM, 4-row groups
  f = psum * scale_c + bias_c            (ScalarE evac -> bf16 FB tiles)
  att = A~^T @ f                         (PE, M=32 zero-padded stationary,
    quadrant-packed into one [128,1024] PSUM tile per 16 f rows via
    tile_position; one [32,1024] ScalarE evac per 4-row group)
  att row-layout repack                  (few strided SBUF-SBUF DMAs)
  z2 = banded-matrix matmuls over rows   (PE, K=2bw+2 -> M=bw, 3 x-shifts;
    pos2 added via identity band rows fed from a resident pos2 tile)
  map = sigmoid(z2)                      (ScalarE, psum -> bf16)
  map broadcast to 128 channel partitions (4 gpsimd DMAs to partitions
    0/32/64/96 + VectorE stream_shuffle)
  out = f * map                          (VectorE bf16*bf16 -> fp32 out tile)
  out DMA per block                      (sync/HWDGE)
Startup: warmup matmuls on a memset tile keep/ramp the PE clock while the
first x slice + weights DMA in. Tail: last blocks are 8/5/4 rows so the final
serial chain after the last conv matmul is short.
"""

import sys
from contextlib import ExitStack

import numpy as np

for _p in ("/opt/trn_rl_repo", "/root/.axon_site/_ro/trn_rl_repo"):
    if _p not in sys.path:
        sys.path.append(_p)

import concourse.bass as bass
import concourse.mybir as mybir
import concourse.tile as tile
from concourse.bass_utils import run_bass_kernel_spmd

F32 = mybir.dt.float32
BF16 = mybir.dt.bfloat16
AF = mybir.ActivationFunctionType

# Problem constants (hardcoded per contract).
B, CIN, COUT, H, W = 4, 128, 128, 256, 256
BR = 64
EPS_BR = 1e-3
EPS_ATT = 1e-5
WP = W + 2            # padded row stride in SBUF
NF = 130              # f rows per core (128 + 1 halo each side)
G = 4                 # f rows per conv group
NGRP = 33             # 32 full groups + 1 tail group of 2 rows
XROWS = 24            # f rows per x input tile (6 conv groups)
NXT = 6               # number of x tiles
# attention blocks in f-row space: (first f row, width). Blocks 0-6 are
# 16-row and aligned with the FB tiles; the tail is 8/5/4 so the final
# serial chain (z2/sigmoid/shuffle/mul/out) after the last conv is short.
BLOCKS = [(0, 16), (16, 16), (32, 16), (48, 16), (64, 16), (80, 16),
          (96, 16), (112, 8), (120, 5), (125, 4)]
NBLK = len(BLOCKS)
BANDK = 18            # band partitions: bw+2 att rows (max)
BANDC = sum(6 * bw for _, bw in BLOCKS)  # 774
NWARM = 28            # PE warmup matmuls at start


def _grp_rows(g):
    """(start_f_row, n_rows) of conv group g."""
    return (G * g, 2 if g == NGRP - 1 else G)


def _band_off(b):
    """Column offset of block b's strip in the band matrix."""
    return sum(6 * bw for _, bw in BLOCKS[:b])


def emit_core(tc, outs, ins):
    """Emit the per-core program. outs/ins are dicts of DRAM APs."""
    nc = tc.nc
    out_d = outs["out"]
    xh_d, wf_d = ins["xh"], ins["wf"]
    chs_d, chb_d, av_d = ins["chs"], ins["chb"], ins["av"]
    band_d, pos2_d, ident_d = ins["band"], ins["pos2"], ins["ident"]
    # round-robin DMA trigger queues to avoid serializing one engine queue
    dmaq = [nc.sync, nc.scalar, nc.gpsimd]
    qi = [0]

    def dma_rr(out_ap, in_ap):
        q = dmaq[qi[0] % 3]
        qi[0] += 1
        q.dma_start(out_ap, in_ap)

    ctx = ExitStack()
    with ctx:
        const = ctx.enter_context(tc.tile_pool(name="const", bufs=1))
        xp = ctx.enter_context(tc.tile_pool(name="xp", bufs=3))
        fbp = ctx.enter_context(tc.tile_pool(name="fbp", bufs=3))
        a32p = ctx.enter_context(tc.tile_pool(name="a32p", bufs=3))
        mapp = ctx.enter_context(tc.tile_pool(name="mapp", bufs=2))
        mrepp = ctx.enter_context(tc.tile_pool(name="mrepp", bufs=2))
        outp = ctx.enter_context(tc.tile_pool(name="outp", bufs=2))
        fps = ctx.enter_context(tc.tile_pool(name="fps", bufs=2, space="PSUM"))
        atps = ctx.enter_context(tc.tile_pool(name="atps", bufs=1, space="PSUM"))
        z2ps = ctx.enter_context(tc.tile_pool(name="z2ps", bufs=2, space="PSUM"))

        # --- startup: memsets (gpsimd), const loads, warmup matmuls -------
        warm = const.tile([128, 512], BF16)
        nc.gpsimd.memset(warm[:], 0.0)

        # x tile 0 first (sync queue), in two pieces so group 0 starts early
        x_t = [None] * NXT
        x_t[0] = xp.tile([CIN, 26, WP], BF16, name="x_t")
        nc.sync.dma_start(x_t[0][:, 0:6, :], xh_d[:, 0:6, :])
        nc.sync.dma_start(x_t[0][:, 6:16, :], xh_d[:, 6:16, :])
        nc.sync.dma_start(x_t[0][:, 16:26, :], xh_d[:, 16:26, :])

        wf_sb = const.tile([CIN, 9 * COUT], BF16)
        nc.scalar.dma_start(wf_sb[:], wf_d[:])
        chs_sb = const.tile([COUT, 1], F32)
        nc.scalar.dma_start(chs_sb[:], chs_d[:])
        chb_sb = const.tile([COUT, 1], F32)
        nc.scalar.dma_start(chb_sb[:], chb_d[:])
        av_sb = const.tile([CIN, 32], BF16)
        nc.scalar.dma_start(av_sb[:], av_d[:])
        band_sb = const.tile([BANDK, BANDC], BF16)
        nc.sync.dma_start(band_sb[:], band_d[:])
        pos2_sb = const.tile([16, NBLK * 512], BF16)
        nc.sync.dma_start(pos2_sb[:], pos2_d[:])
        ident_sb = const.tile([16, 16], BF16)
        nc.gpsimd.dma_start(ident_sb[:], ident_d[:])

        # map staging tiles (partitions 0/32/64/96 hold map values); memset
        # once so stream_shuffle never reads uninitialized SBUF.
        m4 = [const.tile([128, 16 * W], BF16, name=f"m4_{i}") for i in range(2)]
        nc.gpsimd.memset(m4[0][:], 0.0)
        nc.gpsimd.memset(m4[1][:], 0.0)
        # att row-layout tiles, double buffered. Full memset once: the x-pad
        # columns (0/257 of each branch strip) stay zero forever after.
        att_rt = [const.tile([BANDK, 2 * WP], BF16, name=f"att_rt_{i}")
                  for i in range(2)]
        nc.gpsimd.memset(att_rt[0][:], 0.0)
        nc.gpsimd.memset(att_rt[1][:], 0.0)

        # PE warmup: matmuls on the memset tile ramp the clock gate while the
        # first x slice + weights are still in flight.
        wps = fps.tile([COUT, G * W], F32, tag="big", name="f_ps")
        for _ in range(NWARM):
            nc.tensor.matmul(wps[:, 0:128], warm[:, 0:128], warm[:, 0:128],
                             start=True, stop=True)

        FB = [None] * 9              # 16-f-row block -> SBUF f tile (bf16)
        att32 = [None] * 9           # 16-f-row block -> quadrant att tile
        att_ps = [None]              # current att PSUM tile
        att_rows = [0]               # f rows [0, att_rows) have evac'd att
        next_blk = [0]               # next block to emit
        blk_i = [0]                  # att_rt buffer alternation index

        def emit_att_rt(b):
            """Repack att rows + pos2 rows for block b into row layout."""
            o0, bw = BLOCKS[b]
            katt = bw + 2
            rt = att_rt[blk_i[0] % 2]
            blk_i[0] += 1
            # att rows i=0..katt-1  <-  f rows o0-1+i  (f row -1: stays stale
            # /zero; its band taps are zero or feed only the discarded row 0)
            fr = max(o0 - 1, 0)
            fr_end = o0 + katt - 1  # exclusive
            while fr < fr_end:
                beta = fr // 16
                hi = min(fr_end, 16 * (beta + 1), 130)
                # split [fr, hi) into head (to 4-align), middle, tail
                cuts = [fr]
                a = min(hi, (fr + 3) // 4 * 4)
                if a > fr:
                    cuts.append(a)
                m = a + (hi - a) // 4 * 4
                if m > cuts[-1]:
                    cuts.append(m)
                if hi > cuts[-1]:
                    cuts.append(hi)
                for lo, hi2 in zip(cuts[:-1], cuts[1:]):
                    i0 = lo - (o0 - 1)
                    n = hi2 - lo
                    r = lo - 16 * beta
                    for br in range(2):
                        dst = rt[i0:i0 + n, br * WP + 1:br * WP + 1 + W]
                        if r % 4 == 0 and n % 4 == 0:
                            q0, nq = r // 4, n // 4
                            src = att32[beta][32 * q0 + br:
                                              32 * (q0 + nq - 1) + br + 1:32,
                                              0:1024]
                        else:
                            q0, j0 = r // 4, r % 4
                            src = att32[beta][32 * q0 + br:32 * q0 + br + 1,
                                              j0 * W:(j0 + n) * W]
                        dma_rr(dst, src)
                fr = hi
            return rt

        def emit_block(b):
            """z2 banded conv + sigmoid + broadcast + mul + out for block b."""
            o0, bw = BLOCKS[b]
            katt = bw + 2
            rt = emit_att_rt(b)
            cb = _band_off(b)
            z2t = z2ps.tile([16, 512], F32, tag="z2", name="z2t")
            for br in range(2):
                for dx in range(3):
                    nc.tensor.matmul(
                        z2t[0:bw, br * W:(br + 1) * W],
                        band_sb[0:katt, cb + (br * 3 + dx) * bw:cb + (br * 3 + dx + 1) * bw],
                        rt[0:katt, br * WP + dx:br * WP + dx + W],
                        start=(dx == 0), stop=False)
                nc.tensor.matmul(
                    z2t[0:bw, br * W:(br + 1) * W],
                    ident_sb[0:bw, 0:bw],
                    pos2_sb[0:bw, b * 512 + br * W:b * 512 + (br + 1) * W],
                    start=False, stop=True)
            mapS = mapp.tile([16, 512], BF16, name="mapS")
            nc.scalar.activation(mapS[0:bw, :], z2t[0:bw, :], AF.Sigmoid)
            m4_ = m4[b % 2]
            for br, parts in ((0, (0, 32)), (1, (64, 96))):
                src = mapS[0:bw, br * W:(br + 1) * W]
                for eng, p in ((nc.gpsimd, parts[0]), (nc.scalar, parts[1])):
                    eng.dma_start(m4_[p:p + 1, 0:bw * W], src)
            mrep = mrepp.tile([128, 16 * W], BF16, name="mrep")
            nc.vector.stream_shuffle(mrep[:, 0:bw * W], m4_[:, 0:bw * W],
                                     [0] * 32)
            out_t = outp.tile([COUT, 16 * W], F32, name="out_t")
            # multiply against the FB tile(s) covering f rows [o0, o0+bw)
            r = o0
            while r < o0 + bw:
                beta = r // 16
                n = min(o0 + bw - r, 16 * (beta + 1) - r)
                nc.vector.tensor_mul(
                    out_t[:, (r - o0) * W:(r - o0 + n) * W],
                    FB[beta][:, (r - 16 * beta) * W:(r - 16 * beta + n) * W],
                    mrep[:, (r - o0) * W:(r - o0 + n) * W])
                r += n
            lo = max(o0, 1)          # f row 0 is halo, not an output
            hi = o0 + bw
            oq = nc.sync if b % 2 == 0 else nc.scalar
            oq.dma_start(out_d[:, lo - 1:hi - 1, :],
                         out_t[:, (lo - o0) * W:(hi - o0) * W])

        def flush_ready(force=False):
            while next_blk[0] < NBLK:
                b = next_blk[0]
                o0, bw = BLOCKS[b]
                if att_rows[0] < o0 + bw + 1:
                    return
                emit_block(b)
                next_blk[0] += 1

        def emit_att(g):
            """att matmuls for group g into quadrant 32*(g%4) + evac slice."""
            f0, n = _grp_rows(g)
            beta, q = f0 // 16, (f0 % 16) // 4
            if q == 0:
                att_ps[0] = atps.tile([128, G * W], F32, name="att_ps")
                att32[beta] = a32p.tile([128, G * W], BF16, name="att32")
            aps = att_ps[0]
            off = (f0 - 16 * beta) * W
            split = (g == NGRP - 2)  # g31: evac per half so block 9 unblocks
            for j in range(n * W // 512):
                nc.tensor.matmul(
                    aps[32 * q:32 * q + 32, j * 512:(j + 1) * 512],
                    av_sb[:], FB[beta][:, off + j * 512:off + (j + 1) * 512],
                    start=True, stop=True, tile_position=(0, 32 * q))
                if split:
                    nc.scalar.copy(
                        att32[beta][32 * q:32 * q + 32, j * 512:(j + 1) * 512],
                        aps[32 * q:32 * q + 32, j * 512:(j + 1) * 512])
                    att_rows[0] = f0 + 2 * (j + 1)
                    flush_ready(force=(g == NGRP - 1))
            if not split:
                nc.scalar.copy(att32[beta][32 * q:32 * q + 32, 0:n * W],
                               aps[32 * q:32 * q + 32, 0:n * W])
                att_rows[0] = f0 + n
                flush_ready(force=(g == NGRP - 1))

        # ------------------------------------------------------------ main --
        for g in range(NGRP):
            f0, n = _grp_rows(g)
            k = g // 6
            if g % 6 == 1 and (g + 5) // 6 < NXT:
                kk = (g + 5) // 6
                nrows = min(26, 132 - XROWS * kk)
                x_t[kk] = xp.tile([CIN, nrows, WP], BF16, name="x_t")
                nc.sync.dma_start(x_t[kk][:], xh_d[:, XROWS * kk:XROWS * kk + nrows, :])
            xr0 = f0 - XROWS * k
            f_ps = fps.tile([COUT, G * W], F32, tag="big", name="f_ps")
            for tap in range(9):
                ky, kx = tap // 3, tap % 3
                lhsT = wf_sb[:, tap * COUT:(tap + 1) * COUT]
                for j in range(n // 2):
                    rhs = x_t[k][:, xr0 + 2 * j + ky:xr0 + 2 * j + ky + 2,
                                 kx:kx + W]
                    nc.tensor.matmul(
                        f_ps[:, j * 512:(j + 1) * 512], lhsT,
                        rhs, start=(tap == 0), stop=(tap == 8))
            beta = f0 // 16
            if FB[beta] is None:
                FB[beta] = fbp.tile([COUT, 16 * W], BF16, name="FB")
            off = (f0 - 16 * beta) * W
            nc.scalar.activation(FB[beta][:, off:off + n * W], f_ps[:, :n * W],
                                 AF.Identity, bias=chb_sb[:], scale=chs_sb[:])
            if g >= 1:
                emit_att(g - 1)
        emit_att(NGRP - 1)
        flush_ready(force=True)
        assert next_blk[0] == NBLK, f"blocks left: {next_blk[0]}"


# ---------------------------------------------------------------- host side --

def _position_grids():
    i = np.arange(H, dtype=np.float64)
    j = np.arange(W, dtype=np.float64)
    gh = np.abs(i - H // 2 + 0.5) / float(H // 2)
    gw = np.abs(j - W // 2 + 0.5) / float(W // 2)
    GH = np.broadcast_to(gh[:, None], (H, W))
    GW = np.broadcast_to(gw[None, :], (H, W))
    pr = np.sqrt(GH ** 2 + GW ** 2)
    k = 2.0 / (pr.max() - pr.min())
    pr = k * pr + (1.0 - pr.max() * k)
    return GH, GW, pr


def _conv3x3_zp(x, w):
    """x: [C, H, W], w: [O, C, 3, 3] -> [O, H, W] zero-padded conv."""
    C, H_, W_ = x.shape
    O = w.shape[0]
    xp = np.pad(x, ((0, 0), (1, 1), (1, 1)))
    out = np.zeros((O, H_, W_), np.float64)
    for ky in range(3):
        for kx in range(3):
            out += np.einsum("oc,chw->ohw", w[:, :, ky, kx],
                             xp[:, ky:ky + H_, kx:kx + W_])
    return out


def fold_inputs(inp):
    """Host-side constant folding. Returns (shared constants, per-half consts)."""
    import ml_dtypes
    bf = ml_dtypes.bfloat16
    gh, gw, pr = _position_grids()
    Wf = np.zeros((COUT, CIN, 3, 3), np.float64)
    bfv = np.zeros(COUT, np.float64)
    A = np.zeros((CIN, 32), np.float64)
    pos2 = np.zeros((2, H, W), np.float64)
    scales = np.array([float(np.asarray(inp["scale1"])),
                       float(np.asarray(inp["scale2"]))])
    bandw = np.zeros((2, 3, 3), np.float64)
    for bi, br in enumerate("ab"):
        k1 = np.asarray(inp[f"bn_{br}_gamma"], np.float64) / np.sqrt(
            np.asarray(inp[f"bn_{br}_var"], np.float64) + EPS_BR)
        Wf[bi * BR:(bi + 1) * BR] = (
            np.asarray(inp[f"conv_{br}_w"], np.float64) * k1[:, None, None, None])
        bfv[bi * BR:(bi + 1) * BR] = (
            (np.asarray(inp[f"conv_{br}_b"], np.float64)
             - np.asarray(inp[f"bn_{br}_mean"], np.float64)) * k1
            + np.asarray(inp[f"bn_{br}_beta"], np.float64))
        k2 = (float(np.asarray(inp[f"att_bn_{br}_gamma"])[0])
              / np.sqrt(float(np.asarray(inp[f"att_bn_{br}_var"])[0]) + EPS_ATT))
        wa = np.asarray(inp[f"att_{br}_w"], np.float64)[0, :, 0, 0]
        s = scales[bi]
        A[bi * BR:(bi + 1) * BR, bi] = (wa[:BR] * k2 / s) if s != 0.0 else 0.0
        pos1 = (k2 * (wa[BR] * gh + wa[BR + 1] * gw
                      + float(np.asarray(inp[f"att_{br}_b"])[0])
                      - float(np.asarray(inp[f"att_bn_{br}_mean"])[0]))
                + float(np.asarray(inp[f"att_bn_{br}_beta"])[0]))
        attn_w = np.asarray(inp[f"attn_{br}_w"], np.float64)
        pos2[bi] = _conv3x3_zp(np.stack([pos1, gh, gw, pr]), attn_w)[0]
        bandw[bi] = attn_w[0, 0]
    ch_scale = np.repeat(scales, BR)
    shared = {
        # wf DRAM layout: [cin, tap, cout]
        "wf": np.ascontiguousarray(
            Wf.transpose(1, 2, 3, 0).reshape(CIN, 9 * COUT)).astype(bf),
        "chs": ch_scale.reshape(COUT, 1).astype(np.float32),
        "chb": (bfv * ch_scale).reshape(COUT, 1).astype(np.float32),
        "av": A.astype(bf),
    }
    shared["ident"] = np.eye(16).astype(bf)
    halves = []
    for half in range(2):
        r0 = half * 128
        band = np.zeros((BANDK, BANDC), np.float64)
        for b, (o0, bw) in enumerate(BLOCKS):
            base = _band_off(b)
            katt = bw + 2
            for i in range(katt):
                fr_in = o0 - 1 + i
                absr = r0 + fr_in - 1  # abs image row of this att row
                if not (0 <= absr < H):
                    continue
                for o in range(bw):
                    dy = i - o
                    if 0 <= dy <= 2:
                        for brx in range(2):
                            for dx in range(3):
                                band[i, base + (brx * 3 + dx) * bw + o] = \
                                    bandw[brx, dy, dx]
        # pos2_sb: [16, b*512 + br*256 + c] = pos2 at f row o0_b + p
        p2 = np.zeros((16, NBLK * 512), np.float64)
        for b, (o0, bw) in enumerate(BLOCKS):
            for p in range(bw):
                absr = r0 + (o0 + p) - 1
                if not (0 <= absr < H):
                    continue
                for brx in range(2):
                    p2[p, b * 512 + brx * 256:b * 512 + brx * 256 + W] = \
                        pos2[brx, absr]
        halves.append({
            "band": np.ascontiguousarray(band).astype(bf),
            "pos2": np.ascontiguousarray(p2).astype(bf),
        })
    return shared, halves


def make_in_maps(inp):
    import ml_dtypes
    bf = ml_dtypes.bfloat16
    shared, halves = fold_inputs(inp)
    x = np.asarray(inp["x"], np.float32)
    in_maps = []
    for core in range(8):
        b, half = core // 2, core % 2
        r0 = half * 128
        xpad = np.pad(x[b], ((0, 0), (2, 2), (1, 1)))
        xh = np.ascontiguousarray(xpad[:, r0:r0 + 132, :]).astype(bf)
        in_maps.append({"xh": xh, **shared, **halves[half]})
    return in_maps


def _split_matmul_waits(nc):
    """This walrus build accepts only ONE sync wait command per engine
    instruction struct. Move extra waits onto sequencer NoOps inserted just
    before the instruction: the engine queue is processed in order, so the
    sequencer blocks on the NoOp's waits before dispatching it."""
    cnt = 0
    for fn in nc.m.functions:
        for bb in fn.blocks:
            insts = bb.instructions
            i = 0
            while i < len(insts):
                ins = insts[i]
                if (not isinstance(ins, mybir.InstNoOp) and ins.is_executable()
                        and ins.sync_info is not None):
                    w = list(ins.sync_info.on_wait)
                    if len(w) > 1:
                        ins.sync_info = mybir.SyncInfo(
                            on_wait=[w[0]],
                            on_update=list(ins.sync_info.on_update))
                        for sw in w[1:]:
                            cnt += 1
                            nop = mybir.InstNoOp(
                                name=f"I-mmwait-{cnt}", ins=[], outs=[])
                            nop.engine = ins.engine
                            nop.sync_info = mybir.SyncInfo(
                                on_wait=[sw], on_update=[])
                            insts.insert(i, nop)
                            i += 1
                i += 1
    return cnt


_PROGRAM = None


def _build_program():
    global _PROGRAM
    if _PROGRAM is not None:
        return _PROGRAM
    from concourse._compat import axon_active
    nc = bass.Bass("TRN2", target_bir_lowering=False,
                   debug=not axon_active(), enable_asserts=False,
                   num_devices=8)
    ins = {
        "xh": nc.dram_tensor("xh", [CIN, 132, WP], BF16, kind="ExternalInput").ap(),
        "wf": nc.dram_tensor("wf", [CIN, 9 * COUT], BF16, kind="ExternalInput").ap(),
        "chs": nc.dram_tensor("chs", [COUT, 1], F32, kind="ExternalInput").ap(),
        "chb": nc.dram_tensor("chb", [COUT, 1], F32, kind="ExternalInput").ap(),
        "av": nc.dram_tensor("av", [CIN, 32], BF16, kind="ExternalInput").ap(),
        "band": nc.dram_tensor("band", [BANDK, BANDC], BF16,
                               kind="ExternalInput").ap(),
        "pos2": nc.dram_tensor("pos2", [16, NBLK * 512], BF16,
                               kind="ExternalInput").ap(),
        "ident": nc.dram_tensor("ident", [16, 16], BF16,
                                kind="ExternalInput").ap(),
    }
    outs = {
        "out": nc.dram_tensor("out", [COUT, 128, W], F32,
                              kind="ExternalOutput").ap(),
    }
    with tile.TileContext(nc) as tc:
        emit_core(tc, outs, ins)
    _split_matmul_waits(nc)
    _PROGRAM = nc
    return nc


def run_cores(inp, trace=False, **kw):
    """Run the SPMD kernel; returns (full output, BassKernelResults)."""
    nc = _build_program()
    in_maps = make_in_maps(inp)
    res = run_bass_kernel_spmd(nc, in_maps, core_ids=list(range(8)),
                               trace=trace, **kw)
    out = np.zeros((B, COUT, H, W), np.float32)
    for core in range(8):
        b, half = core // 2, core % 2
        out[b, :, half * 128:half * 128 + 128] = res.results[core]["out"]
    return out, res


def kernel(**inputs):
    out, _ = run_cores(inputs)
    return out


# revision 36
# speedup vs baseline: 1.0297x; 1.0297x over previous
"""Trainium2 Bass kernel for nn_AttentionConv2D (two conv3x3+BN branches with
position-aware attention maps), SPMD over 8 NeuronCores.

Sharding: core = batch_index * 2 + h_half. Each core computes both branches for
one batch element's 128-row horizontal slab (plus 1-row halo for the attention
3x3 conv). All cross-core data movement is done host-side (overlapping input
row slices, per-core position/band constants); the device program is identical
on every core.

v2 layout (bf16 input path, fp32 accumulation):
  conv3x3 (both branches, 128 out ch) -> implicit GEMM, 9 taps x row-pair
    matmuls (bf16, N=512) accumulated in PSUM, 4-row groups
  f = psum * scale_c + bias_c            (ScalarE evac -> bf16 FB tiles)
  att = A~^T @ f                         (PE, M=32 zero-padded stationary,
    quadrant-packed into one [128,1024] PSUM tile per 16 f rows via
    tile_position; one [32,1024] ScalarE evac per 4-row group)
  att row-layout repack                  (few strided SBUF-SBUF DMAs)
  z2 = banded-matrix matmuls over rows   (PE, K=2bw+2 -> M=bw, 3 x-shifts;
    pos2 added via identity band rows fed from a resident pos2 tile)
  map = sigmoid(z2)                      (ScalarE, psum -> bf16)
  map broadcast to 128 channel partitions (4 gpsimd DMAs to partitions
    0/32/64/96 + VectorE stream_shuffle)
  out = f * map                          (VectorE bf16*bf16 -> fp32 out tile)
  out DMA per block                      (sync/HWDGE)
Startup: warmup matmuls on a memset tile keep/ramp the PE clock while the
first x slice + weights DMA in. Tail: last blocks are 8/5/4 rows so the final
serial chain after the last conv matmul is short.
"""

import sys
from contextlib import ExitStack

import numpy as np

for _p in ("/opt/trn_rl_repo", "/root/.axon_site/_ro/trn_rl_repo"):
    if _p not in sys.path:
        sys.path.append(_p)

import concourse.bass as bass
import concourse.mybir as mybir
import concourse.tile as tile
from concourse.bass_utils import run_bass_kernel_spmd

F32 = mybir.dt.float32
BF16 = mybir.dt.bfloat16
AF = mybir.ActivationFunctionType

# Problem constants (hardcoded per contract).
B, CIN, COUT, H, W = 4, 128, 128, 256, 256
BR = 64
EPS_BR = 1e-3
EPS_ATT = 1e-5
WP = W + 2            # padded row stride in SBUF
NF = 130              # f rows per core (128 + 1 halo each side)
G = 4                 # f rows per conv group
NGRP = 33             # 32 full groups + 1 tail group of 2 rows
XROWS = 12            # f rows per x input tile (3 conv groups)
NXT = 11              # number of x tiles
# attention blocks in f-row space: (first f row, width). Blocks 0-6 are
# 16-row and aligned with the FB tiles; the tail is 8/5/4 so the final
# serial chain (z2/sigmoid/shuffle/mul/out) after the last conv is short.
BLOCKS = [(0, 16), (16, 16), (32, 16), (48, 16), (64, 16), (80, 16),
          (96, 16), (112, 8), (120, 5), (125, 4)]
NBLK = len(BLOCKS)
BANDK = 18            # band partitions: bw+2 att rows (max)
BANDC = sum(6 * bw for _, bw in BLOCKS)  # 774
NWARM = 34            # PE warmup matmuls at start


def _grp_rows(g):
    """(start_f_row, n_rows) of conv group g."""
    return (G * g, 2 if g == NGRP - 1 else G)


def _band_off(b):
    """Column offset of block b's strip in the band matrix."""
    return sum(6 * bw for _, bw in BLOCKS[:b])


def emit_core(tc, outs, ins):
    """Emit the per-core program. outs/ins are dicts of DRAM APs."""
    nc = tc.nc
    out_d = outs["out"]
    xh_d, wf_d = ins["xh"], ins["wf"]
    chs_d, chb_d, av_d = ins["chs"], ins["chb"], ins["av"]
    band_d, pos2_d, ident_d = ins["band"], ins["pos2"], ins["ident"]
    # round-robin DMA trigger queues to avoid serializing one engine queue
    dmaq = [nc.sync, nc.scalar, nc.gpsimd]
    qi = [0]

    def dma_rr(out_ap, in_ap):
        q = dmaq[qi[0] % 3]
        qi[0] += 1
        q.dma_start(out_ap, in_ap)

    ctx = ExitStack()
    with ctx:
        const = ctx.enter_context(tc.tile_pool(name="const", bufs=1))
        xp = ctx.enter_context(tc.tile_pool(name="xp", bufs=4))
        fbp = ctx.enter_context(tc.tile_pool(name="fbp", bufs=3))
        a32p = ctx.enter_context(tc.tile_pool(name="a32p", bufs=3))
        mapp = ctx.enter_context(tc.tile_pool(name="mapp", bufs=2))
        mrepp = ctx.enter_context(tc.tile_pool(name="mrepp", bufs=3))
        outp = ctx.enter_context(tc.tile_pool(name="outp", bufs=2))
        fps = ctx.enter_context(tc.tile_pool(name="fps", bufs=2, space="PSUM"))
        atps = ctx.enter_context(tc.tile_pool(name="atps", bufs=1, space="PSUM"))
        z2ps = ctx.enter_context(tc.tile_pool(name="z2ps", bufs=2, space="PSUM"))

        # --- startup: memsets (gpsimd), const loads, warmup matmuls -------
        warm = const.tile([128, 512], BF16)
        nc.gpsimd.memset(warm[:], 0.0)

        # x tile 0 first (sync queue), in two pieces so group 0 starts early
        x_t = [None] * NXT
        x_t[0] = xp.tile([CIN, 14, WP], BF16, name="x_t")
        nc.sync.dma_start(x_t[0][:, 0:6, :], xh_d[:, 0:6, :])
        nc.sync.dma_start(x_t[0][:, 6:14, :], xh_d[:, 6:14, :])

        wf_sb = const.tile([CIN, 9 * COUT], BF16)
        nc.scalar.dma_start(wf_sb[:], wf_d[:])
        chs_sb = const.tile([COUT, 1], F32)
        nc.scalar.dma_start(chs_sb[:], chs_d[:])
        chb_sb = const.tile([COUT, 1], F32)
        nc.scalar.dma_start(chb_sb[:], chb_d[:])
        av_sb = const.tile([CIN, 32], BF16)
        nc.scalar.dma_start(av_sb[:], av_d[:])
        band_sb = const.tile([BANDK, BANDC], BF16)
        nc.sync.dma_start(band_sb[:], band_d[:])
        pos2_sb = const.tile([16, NBLK * 512], BF16)
        nc.sync.dma_start(pos2_sb[:], pos2_d[:])
        ident_sb = const.tile([16, 16], BF16)
        nc.gpsimd.dma_start(ident_sb[:], ident_d[:])

        # map staging tiles (partitions 0/32/64/96 hold map values); memset
        # once so stream_shuffle never reads uninitialized SBUF.
        m4 = [const.tile([128, 16 * W], BF16, name=f"m4_{i}") for i in range(2)]
        nc.gpsimd.memset(m4[0][:], 0.0)
        nc.gpsimd.memset(m4[1][:], 0.0)
        # att row-layout tiles, double buffered. Full memset once: the x-pad
        # columns (0/257 of each branch strip) stay zero forever after.
        att_rt = [const.tile([BANDK, 2 * WP], BF16, name=f"att_rt_{i}")
                  for i in range(2)]
        nc.gpsimd.memset(att_rt[0][:], 0.0)
        nc.gpsimd.memset(att_rt[1][:], 0.0)

        # PE warmup: matmuls on the memset tile ramp the clock gate while the
        # first x slice + weights are still in flight.
        wps = fps.tile([COUT, G * W], F32, tag="big", name="f_ps")
        for _ in range(NWARM):
            nc.tensor.matmul(wps[:, 0:128], warm[:, 0:128], warm[:, 0:128],
                             start=True, stop=True)

        FB = [None] * 9              # 16-f-row block -> SBUF f tile (bf16)
        att32 = [None] * 9           # 16-f-row block -> quadrant att tile
        att_ps = [None]              # current att PSUM tile
        att_rows = [0]               # f rows [0, att_rows) have evac'd att
        next_blk = [0]               # next block to emit
        blk_i = [0]                  # att_rt buffer alternation index

        def emit_att_rt(b):
            """Repack att rows + pos2 rows for block b into row layout."""
            o0, bw = BLOCKS[b]
            katt = bw + 2
            rt = att_rt[blk_i[0] % 2]
            blk_i[0] += 1
            # att rows i=0..katt-1  <-  f rows o0-1+i  (f row -1: stays stale
            # /zero; its band taps are zero or feed only the discarded row 0)
            fr = max(o0 - 1, 0)
            fr_end = o0 + katt - 1  # exclusive
            while fr < fr_end:
                beta = fr // 16
                hi = min(fr_end, 16 * (beta + 1), 130)
                # split [fr, hi) into head (to 4-align), middle, tail
                cuts = [fr]
                a = min(hi, (fr + 3) // 4 * 4)
                if a > fr:
                    cuts.append(a)
                m = a + (hi - a) // 4 * 4
                if m > cuts[-1]:
                    cuts.append(m)
                if hi > cuts[-1]:
                    cuts.append(hi)
                for lo, hi2 in zip(cuts[:-1], cuts[1:]):
                    i0 = lo - (o0 - 1)
                    n = hi2 - lo
                    r = lo - 16 * beta
                    for br in range(2):
                        dst = rt[i0:i0 + n, br * WP + 1:br * WP + 1 + W]
                        if r % 4 == 0 and n % 4 == 0:
                            q0, nq = r // 4, n // 4
                            src = att32[beta][32 * q0 + br:
                                              32 * (q0 + nq - 1) + br + 1:32,
                                              0:1024]
                        else:
                            q0, j0 = r // 4, r % 4
                            src = att32[beta][32 * q0 + br:32 * q0 + br + 1,
                                              j0 * W:(j0 + n) * W]
                        dma_rr(dst, src)
                fr = hi
            return rt

        def emit_block(b):
            """z2 banded conv + sigmoid + broadcast + mul + out for block b."""
            o0, bw = BLOCKS[b]
            katt = bw + 2
            rt = emit_att_rt(b)
            cb = _band_off(b)
            z2t = z2ps.tile([16, 512], F32, tag="z2", name="z2t")
            for br in range(2):
                for dx in range(3):
                    nc.tensor.matmul(
                        z2t[0:bw, br * W:(br + 1) * W],
                        band_sb[0:katt, cb + (br * 3 + dx) * bw:cb + (br * 3 + dx + 1) * bw],
                        rt[0:katt, br * WP + dx:br * WP + dx + W],
                        start=(dx == 0), stop=False)
                nc.tensor.matmul(
                    z2t[0:bw, br * W:(br + 1) * W],
                    ident_sb[0:bw, 0:bw],
                    pos2_sb[0:bw, b * 512 + br * W:b * 512 + (br + 1) * W],
                    start=False, stop=True)
            mapS = mapp.tile([16, 512], BF16, name="mapS")
            nc.scalar.activation(mapS[0:bw, :], z2t[0:bw, :], AF.Sigmoid)
            m4_ = m4[b % 2]
            tailq = ((nc.gpsimd, 0), (nc.scalar, 32), (nc.sync, 64),
                     (nc.gpsimd, 96)) if b >= NBLK - 2 else                     ((nc.gpsimd, 0), (nc.scalar, 32), (nc.gpsimd, 64),
                     (nc.scalar, 96))
            for eng, p in tailq:
                br = 0 if p < 64 else 1
                eng.dma_start(m4_[p:p + 1, 0:bw * W],
                              mapS[0:bw, br * W:(br + 1) * W])
            mrep = mrepp.tile([128, 16 * W], BF16, name="mrep")
            nc.vector.stream_shuffle(mrep[:, 0:bw * W], m4_[:, 0:bw * W],
                                     [0] * 32)
            out_t = outp.tile([COUT, 16 * W], F32, name="out_t")
            # multiply against the FB tile(s) covering f rows [o0, o0+bw)
            r = o0
            while r < o0 + bw:
                beta = r // 16
                n = min(o0 + bw - r, 16 * (beta + 1) - r)
                nc.vector.tensor_mul(
                    out_t[:, (r - o0) * W:(r - o0 + n) * W],
                    FB[beta][:, (r - 16 * beta) * W:(r - 16 * beta + n) * W],
                    mrep[:, (r - o0) * W:(r - o0 + n) * W])
                r += n
            lo = max(o0, 1)          # f row 0 is halo, not an output
            hi = o0 + bw
            oq = nc.sync if b % 2 == 0 else nc.scalar
            oq.dma_start(out_d[:, lo - 1:hi - 1, :],
                         out_t[:, (lo - o0) * W:(hi - o0) * W])

        def flush_ready(force=False):
            while next_blk[0] < NBLK:
                b = next_blk[0]
                o0, bw = BLOCKS[b]
                if att_rows[0] < o0 + bw + 1:
                    return
                emit_block(b)
                next_blk[0] += 1

        def emit_att(g):
            """att matmuls for group g into quadrant 32*(g%4) + evac slice."""
            f0, n = _grp_rows(g)
            beta, q = f0 // 16, (f0 % 16) // 4
            if q == 0:
                att_ps[0] = atps.tile([128, G * W], F32, name="att_ps")
                att32[beta] = a32p.tile([128, G * W], BF16, name="att32")
            aps = att_ps[0]
            off = (f0 - 16 * beta) * W
            split = (g == NGRP - 2)  # g31: evac per half so block 9 unblocks
            for j in range(n * W // 512):
                nc.tensor.matmul(
                    aps[32 * q:32 * q + 32, j * 512:(j + 1) * 512],
                    av_sb[:], FB[beta][:, off + j * 512:off + (j + 1) * 512],
                    start=True, stop=True, tile_position=(0, 32 * q))
                if split:
                    nc.scalar.copy(
                        att32[beta][32 * q:32 * q + 32, j * 512:(j + 1) * 512],
                        aps[32 * q:32 * q + 32, j * 512:(j + 1) * 512])
                    att_rows[0] = f0 + 2 * (j + 1)
                    flush_ready(force=(g == NGRP - 1))
            if not split:
                nc.scalar.copy(att32[beta][32 * q:32 * q + 32, 0:n * W],
                               aps[32 * q:32 * q + 32, 0:n * W])
                att_rows[0] = f0 + n
                flush_ready(force=(g == NGRP - 1))

        # ------------------------------------------------------------ main --
        for g in range(NGRP):
            f0, n = _grp_rows(g)
            k = g // 3
            if g % 3 == 1 and (g + 2) // 3 < NXT:
                kk = (g + 2) // 3
                nrows = min(14, 132 - XROWS * kk)
                x_t[kk] = xp.tile([CIN, nrows, WP], BF16, name="x_t")
                nc.sync.dma_start(x_t[kk][:], xh_d[:, XROWS * kk:XROWS * kk + nrows, :])
            xr0 = f0 - XROWS * k
            f_ps = fps.tile([COUT, G * W], F32, tag="big", name="f_ps")
            for tap in range(9):
                ky, kx = tap // 3, tap % 3
                lhsT = wf_sb[:, tap * COUT:(tap + 1) * COUT]
                for j in range(n // 2):
                    rhs = x_t[k][:, xr0 + 2 * j + ky:xr0 + 2 * j + ky + 2,
                                 kx:kx + W]
                    nc.tensor.matmul(
                        f_ps[:, j * 512:(j + 1) * 512], lhsT,
                        rhs, start=(tap == 0), stop=(tap == 8))
            beta = f0 // 16
            if FB[beta] is None:
                FB[beta] = fbp.tile([COUT, 16 * W], BF16, name="FB")
            off = (f0 - 16 * beta) * W
            nc.scalar.activation(FB[beta][:, off:off + n * W], f_ps[:, :n * W],
                                 AF.Identity, bias=chb_sb[:], scale=chs_sb[:])
            if g >= 1:
                emit_att(g - 1)
        emit_att(NGRP - 1)
        flush_ready(force=True)
        assert next_blk[0] == NBLK, f"blocks left: {next_blk[0]}"


# ---------------------------------------------------------------- host side --

def _position_grids():
    i = np.arange(H, dtype=np.float64)
    j = np.arange(W, dtype=np.float64)
    gh = np.abs(i - H // 2 + 0.5) / float(H // 2)
    gw = np.abs(j - W // 2 + 0.5) / float(W // 2)
    GH = np.broadcast_to(gh[:, None], (H, W))
    GW = np.broadcast_to(gw[None, :], (H, W))
    pr = np.sqrt(GH ** 2 + GW ** 2)
    k = 2.0 / (pr.max() - pr.min())
    pr = k * pr + (1.0 - pr.max() * k)
    return GH, GW, pr


def _conv3x3_zp(x, w):
    """x: [C, H, W], w: [O, C, 3, 3] -> [O, H, W] zero-padded conv."""
    C, H_, W_ = x.shape
    O = w.shape[0]
    xp = np.pad(x, ((0, 0), (1, 1), (1, 1)))
    out = np.zeros((O, H_, W_), np.float64)
    for ky in range(3):
        for kx in range(3):
            out += np.einsum("oc,chw->ohw", w[:, :, ky, kx],
                             xp[:, ky:ky + H_, kx:kx + W_])
    return out


def fold_inputs(inp):
    """Host-side constant folding. Returns (shared constants, per-half consts)."""
    import ml_dtypes
    bf = ml_dtypes.bfloat16
    gh, gw, pr = _position_grids()
    Wf = np.zeros((COUT, CIN, 3, 3), np.float64)
    bfv = np.zeros(COUT, np.float64)
    A = np.zeros((CIN, 32), np.float64)
    pos2 = np.zeros((2, H, W), np.float64)
    scales = np.array([float(np.asarray(inp["scale1"])),
                       float(np.asarray(inp["scale2"]))])
    bandw = np.zeros((2, 3, 3), np.float64)
    for bi, br in enumerate("ab"):
        k1 = np.asarray(inp[f"bn_{br}_gamma"], np.float64) / np.sqrt(
            np.asarray(inp[f"bn_{br}_var"], np.float64) + EPS_BR)
        Wf[bi * BR:(bi + 1) * BR] = (
            np.asarray(inp[f"conv_{br}_w"], np.float64) * k1[:, None, None, None])
        bfv[bi * BR:(bi + 1) * BR] = (
            (np.asarray(inp[f"conv_{br}_b"], np.float64)
             - np.asarray(inp[f"bn_{br}_mean"], np.float64)) * k1
            + np.asarray(inp[f"bn_{br}_beta"], np.float64))
        k2 = (float(np.asarray(inp[f"att_bn_{br}_gamma"])[0])
              / np.sqrt(float(np.asarray(inp[f"att_bn_{br}_var"])[0]) + EPS_ATT))
        wa = np.asarray(inp[f"att_{br}_w"], np.float64)[0, :, 0, 0]
        s = scales[bi]
        A[bi * BR:(bi + 1) * BR, bi] = (wa[:BR] * k2 / s) if s != 0.0 else 0.0
        pos1 = (k2 * (wa[BR] * gh + wa[BR + 1] * gw
                      + float(np.asarray(inp[f"att_{br}_b"])[0])
                      - float(np.asarray(inp[f"att_bn_{br}_mean"])[0]))
                + float(np.asarray(inp[f"att_bn_{br}_beta"])[0]))
        attn_w = np.asarray(inp[f"attn_{br}_w"], np.float64)
        pos2[bi] = _conv3x3_zp(np.stack([pos1, gh, gw, pr]), attn_w)[0]
        bandw[bi] = attn_w[0, 0]
    ch_scale = np.repeat(scales, BR)
    shared = {
        # wf DRAM layout: [cin, tap, cout]
        "wf": np.ascontiguousarray(
            Wf.transpose(1, 2, 3, 0).reshape(CIN, 9 * COUT)).astype(bf),
        "chs": ch_scale.reshape(COUT, 1).astype(np.float32),
        "chb": (bfv * ch_scale).reshape(COUT, 1).astype(np.float32),
        "av": A.astype(bf),
    }
    shared["ident"] = np.eye(16).astype(bf)
    halves = []
    for half in range(2):
        r0 = half * 128
        band = np.zeros((BANDK, BANDC), np.float64)
        for b, (o0, bw) in enumerate(BLOCKS):
            base = _band_off(b)
            katt = bw + 2
            for i in range(katt):
                fr_in = o0 - 1 + i
                absr = r0 + fr_in - 1  # abs image row of this att row
                if not (0 <= absr < H):
                    continue
                for o in range(bw):
                    dy = i - o
                    if 0 <= dy <= 2:
                        for brx in range(2):
                            for dx in range(3):
                                band[i, base + (brx * 3 + dx) * bw + o] = \
                                    bandw[brx, dy, dx]
        # pos2_sb: [16, b*512 + br*256 + c] = pos2 at f row o0_b + p
        p2 = np.zeros((16, NBLK * 512), np.float64)
        for b, (o0, bw) in enumerate(BLOCKS):
            for p in range(bw):
                absr = r0 + (o0 + p) - 1
                if not (0 <= absr < H):
                    continue
                for brx in range(2):
                    p2[p, b * 512 + brx * 256:b * 512 + brx * 256 + W] = \
                        pos2[brx, absr]
        halves.append({
            "band": np.ascontiguousarray(band).astype(bf),
            "pos2": np.ascontiguousarray(p2).astype(bf),
        })
    return shared, halves


def make_in_maps(inp):
    import ml_dtypes
    bf = ml_dtypes.bfloat16
    shared, halves = fold_inputs(inp)
    x = np.asarray(inp["x"], np.float32)
    in_maps = []
    for core in range(8):
        b, half = core // 2, core % 2
        r0 = half * 128
        xpad = np.pad(x[b], ((0, 0), (2, 2), (1, 1)))
        xh = np.ascontiguousarray(xpad[:, r0:r0 + 132, :]).astype(bf)
        in_maps.append({"xh": xh, **shared, **halves[half]})
    return in_maps


def _split_matmul_waits(nc):
    """This walrus build accepts only ONE sync wait command per engine
    instruction struct. Move extra waits onto sequencer NoOps inserted just
    before the instruction: the engine queue is processed in order, so the
    sequencer blocks on the NoOp's waits before dispatching it."""
    cnt = 0
    for fn in nc.m.functions:
        for bb in fn.blocks:
            insts = bb.instructions
            i = 0
            while i < len(insts):
                ins = insts[i]
                if (not isinstance(ins, mybir.InstNoOp) and ins.is_executable()
                        and ins.sync_info is not None):
                    w = list(ins.sync_info.on_wait)
                    if len(w) > 1:
                        ins.sync_info = mybir.SyncInfo(
                            on_wait=[w[0]],
                            on_update=list(ins.sync_info.on_update))
                        for sw in w[1:]:
                            cnt += 1
                            nop = mybir.InstNoOp(
                                name=f"I-mmwait-{cnt}", ins=[], outs=[])
                            nop.engine = ins.engine
                            nop.sync_info = mybir.SyncInfo(
                                on_wait=[sw], on_update=[])
                            insts.insert(i, nop)
                            i += 1
                i += 1
    return cnt


_PROGRAM = None


def _build_program():
    global _PROGRAM
    if _PROGRAM is not None:
        return _PROGRAM
    from concourse._compat import axon_active
    nc = bass.Bass("TRN2", target_bir_lowering=False,
                   debug=not axon_active(), enable_asserts=False,
                   num_devices=8)
    ins = {
        "xh": nc.dram_tensor("xh", [CIN, 132, WP], BF16, kind="ExternalInput").ap(),
        "wf": nc.dram_tensor("wf", [CIN, 9 * COUT], BF16, kind="ExternalInput").ap(),
        "chs": nc.dram_tensor("chs", [COUT, 1], F32, kind="ExternalInput").ap(),
        "chb": nc.dram_tensor("chb", [COUT, 1], F32, kind="ExternalInput").ap(),
        "av": nc.dram_tensor("av", [CIN, 32], BF16, kind="ExternalInput").ap(),
        "band": nc.dram_tensor("band", [BANDK, BANDC], BF16,
                               kind="ExternalInput").ap(),
        "pos2": nc.dram_tensor("pos2", [16, NBLK * 512], BF16,
                               kind="ExternalInput").ap(),
        "ident": nc.dram_tensor("ident", [16, 16], BF16,
                                kind="ExternalInput").ap(),
    }
    outs = {
        "out": nc.dram_tensor("out", [COUT, 128, W], F32,
                              kind="ExternalOutput").ap(),
    }
    with tile.TileContext(nc) as tc:
        emit_core(tc, outs, ins)
    _split_matmul_waits(nc)
    _PROGRAM = nc
    return nc


def run_cores(inp, trace=False, **kw):
    """Run the SPMD kernel; returns (full output, BassKernelResults)."""
    nc = _build_program()
    in_maps = make_in_maps(inp)
    res = run_bass_kernel_spmd(nc, in_maps, core_ids=list(range(8)),
                               trace=trace, **kw)
    out = np.zeros((B, COUT, H, W), np.float32)
    for core in range(8):
        b, half = core // 2, core % 2
        out[b, :, half * 128:half * 128 + 128] = res.results[core]["out"]
    return out, res


def kernel(**inputs):
    out, _ = run_cores(inputs)
    return out
